# revision 1
# baseline (speedup 1.0000x reference)
"""3-layer GCN encoder on 8 Trainium2 NeuronCores.

Strategy:
- Nodes sharded across 8 cores (6250 real nodes each, padded to 6272 = 49*128
  slots); edges partitioned by destination core.
- GCN norm is symmetric (norm = dinv[src]*dinv[dst]), so node rows in the
  gather table are pre-scaled by dinv; aggregation is a plain sum of gathered
  rows; the result is post-scaled by dinv[dst].
- Aggregate-then-transform: A_hat @ (h W) == (A_hat @ h) W, so every gather
  moves 64-dim (256B) rows regardless of layer.
- Scatter-add is eliminated on-device: each core's destination nodes are
  degree-sorted into 128-node blocks; every node's edge list is padded to the
  block max degree k_b (pad slots point at an all-zero table row). Gather
  indices are laid out slot-major so edge j of block-node p lands at
  (partition p, chunk j) — segment-sum becomes a fixed-shape strided
  tensor_reduce per block.
- dma_gather indices are signed int16, so the 50184-row table is addressed
  through two windows: L = rows of cores 0-4 (31365 rows), H = rows of cores
  5-7 (18819 rows). L and H edge sets get independent degree-sorted layouts;
  the H partial accumulator is realigned to L order via a small 6272-row
  gather through a DRAM scratch buffer.
- Inter-layer exchange: each core AllGathers its 6273-row shard (6272 outputs
  + 1 zero row) into the next layer's replicated table.
"""

import numpy as np


def _install_ntff_hook_shim():
    """The axon boot registers its NTFF profile hook via
    ``antenv.axon_hooks`` — a module this image's antenv package lacks.
    Pre-seed an equivalent holder module so trace=True can profile.
    Must run before jax initializes the axon platform."""
    import sys
    import types

    if "antenv.axon_hooks" in sys.modules:
        return
    mod = types.ModuleType("antenv.axon_hooks")
    holder = [None]
    mod.set_axon_ntff_profile_hook = lambda h: holder.__setitem__(0, h)
    mod.get_axon_ntff_profile_hook = lambda: holder[0]
    sys.modules["antenv.axon_hooks"] = mod
    try:
        import antenv

        antenv.axon_hooks = mod
    except ImportError:
        pass


_install_ntff_hook_shim()

N = 50000
E = 800000
D = 64
DOUT = 32
C = 8
NPC = 6250            # real nodes per core
SLOTS = 6272          # padded slots per core = 49*128
B = 49                # dst blocks per core
SHARD = 6273          # table rows per core (slots + 1 zero row)
L_CORES = 5
L_WIN = L_CORES * SHARD          # 31365
H_WIN = (C - L_CORES) * SHARD    # 18819
TROWS = C * SHARD                # 50184
ZROW = SLOTS                     # zero-row offset within each shard
PIECE_CAP = 8192                 # rows per gather buffer (whole blocks)
GCALL = 1024                     # max rows per dma_gather call (SWDGE
                                 # descriptor-carveout limit: 1024 descs)

_last_results = None  # BassKernelResults of the most recent run (for test.py)


def _wrap_idx(stream):
    """int32 stream -> int16 [128, len/16] wrapped-and-replicated index tile."""
    n = stream.shape[-1]
    assert n % 16 == 0
    w = stream.reshape(-1, n // 16, 16)            # [C?, n/16, 16]
    w = np.swapaxes(w, -1, -2)                     # [..., 16, n/16]
    w = np.tile(w, (1, 8, 1)) if w.ndim == 3 else np.tile(w, (8, 1))
    return np.ascontiguousarray(w).astype(np.int16)


def _slot_layout(dloc, q, kb):
    """Build per-core slot-major index streams.

    dloc: [C, NPC] per-node local degree (in this window), in node-id order
    q:    [C, NPC] per-node position in this window's sorted order
    kb:   [B] shared block slot counts
    Returns (off, positions builder helpers) — used by caller.
    """
    off = np.zeros(B + 1, np.int64)
    off[1:] = np.cumsum(kb)
    return off


def _prep(x, edge_index, W1, b1, W2, b2, W3, b3):
    src = np.asarray(edge_index[0], dtype=np.int64)
    dst = np.asarray(edge_index[1], dtype=np.int64)
    loop = np.arange(N, dtype=np.int64)
    src = np.concatenate([src, loop])
    dst = np.concatenate([dst, loop])

    deg = np.bincount(dst, minlength=N)
    dinv = np.zeros(N, np.float64)
    nz = deg > 0
    dinv[nz] = 1.0 / np.sqrt(deg[nz].astype(np.float64))
    dinv = dinv.astype(np.float32)

    src_core = src // NPC
    is_L = src_core < L_CORES
    d0 = np.bincount(dst[is_L], minlength=N)
    d1 = deg - d0

    # Per-core sorted orders. q0[n] = position of node n in its core's
    # L-order (also its table-row offset); q1[n] = position in H-order.
    q0 = np.empty(N, np.int64)
    q1 = np.empty(N, np.int64)
    k0 = np.zeros(B, np.int64)
    k1 = np.zeros(B, np.int64)
    for c in range(C):
        nodes = np.arange(c * NPC, (c + 1) * NPC)
        o0 = nodes[np.argsort(d0[nodes], kind="stable")]
        o1 = nodes[np.argsort(d1[nodes], kind="stable")]
        q0[o0] = np.arange(NPC)
        q1[o1] = np.arange(NPC)
        p0 = np.zeros(SLOTS, np.int64)
        p0[:NPC] = d0[o0]
        p1 = np.zeros(SLOTS, np.int64)
        p1[:NPC] = d1[o1]
        k0 = np.maximum(k0, p0.reshape(B, 128).max(axis=1))
        k1 = np.maximum(k1, p1.reshape(B, 128).max(axis=1))
    k0 = k0.astype(int)
    k1 = k1.astype(int)
    off0 = np.zeros(B + 1, np.int64)
    off0[1:] = np.cumsum(k0)
    off1 = np.zeros(B + 1, np.int64)
    off1[1:] = np.cumsum(k1)
    L_len = int(128 * off0[-1])
    H_len = int(128 * off1[-1])

    trow = (np.arange(N) // NPC) * SHARD + q0  # table row of each node

    # --- index streams -------------------------------------------------
    def build_streams(sel, qx, offx, length, base):
        """sel: edge mask for this window; qx: dst position array;
        offx: block offsets; base: subtracted from src table row."""
        es, ed = src[sel], dst[sel]
        core = ed // NPC
        qd = qx[ed]
        order = np.argsort(core * SLOTS + qd, kind="stable")
        es, ed, core, qd = es[order], ed[order], core[order], qd[order]
        # rank j of each edge within its destination's list
        key = core * SLOTS + qd
        starts = np.searchsorted(key, key)  # first occurrence index per key
        j = np.arange(len(key)) - starts
        b = qd // 128
        p = qd % 128
        pos = (offx[b] + j) * 128 + p
        streams = np.full((C, length), ZROW, np.int32)
        streams[core, pos] = (trow[es] - base).astype(np.int32)
        return streams

    sL = build_streams(is_L, q0, off0, L_len, 0)
    sH = build_streams(~is_L, q1, off1, H_len, L_WIN)

    idxL = _wrap_idx(sL)            # [C, 128, L_len/16] int16
    idxH = _wrap_idx(sH)

    # --- realign: for L-position i, the scratch row (p1*B + b1) ---------
    idxR = np.empty((C, SLOTS), np.int32)
    for c in range(C):
        nodes = np.arange(c * NPC, (c + 1) * NPC)
        r = np.full(SLOTS, 0, np.int32)
        qq0 = q0[nodes]
        qq1 = q1[nodes]
        r[qq0] = ((qq1 % 128) * B + qq1 // 128).astype(np.int32)
        dummy = np.arange(NPC, SLOTS)
        r[dummy] = ((dummy % 128) * B + dummy // 128).astype(np.int32)
        idxR[c] = r
    idxRw = _wrap_idx(idxR)

    # --- dinv tiles [C, 128, B]: value at (p, b) = dinv(node at q0=b*128+p)
    dinv_t = np.zeros((C, 128, B), np.float32)
    for c in range(C):
        nodes = np.arange(c * NPC, (c + 1) * NPC)
        arr = np.zeros(SLOTS, np.float32)
        arr[q0[nodes]] = dinv[nodes]
        dinv_t[c] = arr.reshape(B, 128).T

    # --- initial table: prescaled, permuted x ---------------------------
    x = np.asarray(x, np.float32)
    xt = np.zeros((TROWS, D), np.float32)
    xt[trow] = x * dinv[:, None]

    meta = dict(k0=k0, k1=k1, off0=off0, off1=off1, L_len=L_len, H_len=H_len)
    host = dict(
        x_table=xt,
        idxL=idxL, idxH=idxH, idxR=idxRw, dinv_t=dinv_t,
        W1=np.asarray(W1, np.float32), W2=np.asarray(W2, np.float32),
        W3=np.asarray(W3, np.float32),
        b1=np.asarray(b1, np.float32).reshape(D, 1),
        b2=np.asarray(b2, np.float32).reshape(D, 1),
        b3=np.asarray(b3, np.float32).reshape(DOUT, 1),
        q0=q0, trow=trow,
    )
    return meta, host


def _pieces(kb, off):
    """Group blocks into pieces with <= PIECE_CAP gathered rows each.
    Returns list of (b_start, b_end, row_off, rows)."""
    out = []
    bs = 0
    while bs < B:
        be = bs
        rows = 0
        while be < B and (rows + 128 * kb[be]) <= PIECE_CAP:
            rows += 128 * kb[be]
            be += 1
        if be == bs:  # single oversized block
            rows = 128 * kb[bs]
            be = bs + 1
        out.append((bs, be, int(128 * off[bs]), int(rows)))
        bs = be
    return out


def _build(meta):
    import concourse.bacc as bacc
    import concourse.mybir as mybir
    from concourse.tile import TileContext

    k0, k1 = meta["k0"], meta["k1"]
    off0, off1 = meta["off0"], meta["off1"]
    L_len, H_len = meta["L_len"], meta["H_len"]
    f32 = mybir.dt.float32
    i16 = mybir.dt.int16

    nc = bacc.Bacc(None, target_bir_lowering=False, num_swdge_queues=4)

    x_table = nc.declare_dram_parameter("x_table", [TROWS, D], f32, isOutput=False)
    idxL_p = nc.declare_dram_parameter("idxL", [128, L_len // 16], i16, isOutput=False)
    idxH_p = nc.declare_dram_parameter("idxH", [128, H_len // 16], i16, isOutput=False)
    idxR_p = nc.declare_dram_parameter("idxR", [128, SLOTS // 16], i16, isOutput=False)
    dinv_p = nc.declare_dram_parameter("dinv_t", [128, B], f32, isOutput=False)
    W_p = [nc.declare_dram_parameter(f"W{i+1}", [D, D if i < 2 else DOUT], f32, isOutput=False) for i in range(3)]
    b_p = [nc.declare_dram_parameter(f"b{i+1}", [D if i < 2 else DOUT, 1], f32, isOutput=False) for i in range(3)]
    z_ext = nc.declare_dram_parameter("z", [SLOTS, DOUT], f32, isOutput=True)

    tables = [x_table]
    agins = []
    scratches = []
    for l in range(2):
        tables.append(nc.dram_tensor(f"table{l+1}", [TROWS, D], f32, addr_space="Shared"))
        agins.append(nc.dram_tensor(f"agin{l}", [SHARD, D], f32))
    for l in range(3):
        scratches.append(nc.dram_tensor(f"scratch{l}", [SLOTS, D], f32))

    piecesL = _pieces(k0, off0)
    piecesH = _pieces(k1, off1)
    qctr = [0]

    def next_q():
        q = qctr[0] % 4
        qctr[0] += 1
        return q

    with TileContext(nc) as tc:
        with (
            tc.tile_pool(name="const", bufs=1) as cpool,
            tc.tile_pool(name="acc", bufs=1) as apool,
            tc.tile_pool(name="gath", bufs=2) as gpool,
            tc.tile_pool(name="stage", bufs=4) as spool,
            tc.tile_pool(name="psum", bufs=2, space="PSUM") as ppool,
        ):
            # ---- persistent constants ----
            idxL_t = cpool.tile([128, L_len // 16], i16, tag="idxL")
            idxH_t = cpool.tile([128, H_len // 16], i16, tag="idxH")
            idxR_t = cpool.tile([128, SLOTS // 16], i16, tag="idxR")
            dinv_t = cpool.tile([128, B], f32, tag="dinv")
            ident = cpool.tile([128, 128], f32, tag="ident")
            zrow = cpool.tile([1, D], f32, tag="zrow")
            Wt = [cpool.tile([D, D if i < 2 else DOUT], f32, tag=f"W{i}", name=f"Wt{i}") for i in range(3)]
            bt = [cpool.tile([D if i < 2 else DOUT, 1], f32, tag=f"b{i}", name=f"bt{i}") for i in range(3)]

            nc.sync.dma_start(out=idxL_t[:], in_=idxL_p[:])
            nc.sync.dma_start(out=idxH_t[:], in_=idxH_p[:])
            nc.sync.dma_start(out=idxR_t[:], in_=idxR_p[:])
            nc.sync.dma_start(out=dinv_t[:], in_=dinv_p[:])
            for i in range(3):
                nc.sync.dma_start(out=Wt[i][:], in_=W_p[i][:])
                nc.sync.dma_start(out=bt[i][:], in_=b_p[i][:])
            nc.gpsimd.memset(ident[:], 1.0)
            nc.gpsimd.affine_select(
                out=ident[:], in_=ident[:], pattern=[[-1, 128]], base=0,
                channel_multiplier=1, compare_op=mybir.AluOpType.is_equal, fill=0.0)
            nc.vector.memset(zrow[:], 0.0)
            for l in range(2):
                nc.sync.dma_start(out=agins[l][ZROW:ZROW + 1, :], in_=zrow[:])

            # ---- layers ----
            for l in range(3):
                table = tables[l]
                Dl = D if l < 2 else DOUT
                acc0 = apool.tile([128, B, D], f32, tag="acc0")
                acc1 = apool.tile([128, B, D], f32, tag="acc1")
                accR = apool.tile([128, B, D], f32, tag="accR")

                # H phase first (feeds scratch -> realign gather)
                for (bs, be, roff, rows) in piecesH:
                    gh = gpool.tile([128, rows // 128, D], f32, tag="gh")
                    for s0 in range(0, rows, GCALL):
                        sn = min(GCALL, rows - s0)
                        nc.gpsimd.dma_gather(
                            out_ap=gh[:, s0 // 128:(s0 + sn) // 128, :],
                            in_ap=table[L_WIN:TROWS, :],
                            idxs_ap=idxH_t[:, (roff + s0) // 16:(roff + s0 + sn) // 16],
                            num_idxs=sn, num_idxs_reg=sn, elem_size=D,
                            queue_num=next_q())
                    for b in range(bs, be):
                        o = int(128 * (off1[b] - off1[bs])) // 128
                        kb = int(k1[b])
                        nc.vector.tensor_reduce(
                            out=acc1[:, b, :],
                            in_=gh[:, o:o + kb, :].rearrange("p k d -> p d k"),
                            axis=mybir.AxisListType.X, op=mybir.AluOpType.add)
                # acc1 -> scratch (p-major mirror), then realign gather
                nc.sync.dma_start(
                    out=scratches[l][:].rearrange("(p b) d -> p b d", p=128),
                    in_=acc1[:])
                for s0 in range(0, SLOTS, GCALL):
                    sn = min(GCALL, SLOTS - s0)
                    nc.gpsimd.dma_gather(
                        out_ap=accR[:, s0 // 128:(s0 + sn) // 128, :],
                        in_ap=scratches[l][:],
                        idxs_ap=idxR_t[:, s0 // 16:(s0 + sn) // 16],
                        num_idxs=sn, num_idxs_reg=sn, elem_size=D,
                        queue_num=next_q())

                # L phase
                for (bs, be, roff, rows) in piecesL:
                    gl = gpool.tile([128, rows // 128, D], f32, tag="gl")
                    for s0 in range(0, rows, GCALL):
                        sn = min(GCALL, rows - s0)
                        nc.gpsimd.dma_gather(
                            out_ap=gl[:, s0 // 128:(s0 + sn) // 128, :],
                            in_ap=table[0:L_WIN, :],
                            idxs_ap=idxL_t[:, (roff + s0) // 16:(roff + s0 + sn) // 16],
                            num_idxs=sn, num_idxs_reg=sn, elem_size=D,
                            queue_num=next_q())
                    for b in range(bs, be):
                        o = int(128 * (off0[b] - off0[bs])) // 128
                        kb = int(k0[b])
                        nc.vector.tensor_reduce(
                            out=acc0[:, b, :],
                            in_=gl[:, o:o + kb, :].rearrange("p k d -> p d k"),
                            axis=mybir.AxisListType.X, op=mybir.AluOpType.add)

                # output stage per block
                for b in range(B):
                    tot = spool.tile([128, D], f32, tag="tot")
                    nc.vector.tensor_add(tot[:], acc0[:, b, :], accR[:, b, :])
                    scaled = spool.tile([128, D], f32, tag="scaled")
                    nc.scalar.activation(
                        out=scaled[:], in_=tot[:],
                        func=mybir.ActivationFunctionType.Copy,
                        scale=dinv_t[:, b:b + 1])
                    pT = ppool.tile([D, 128], f32, tag="pT")
                    nc.tensor.transpose(pT[:], scaled[:], ident[:])
                    accT = spool.tile([D, 128], f32, tag="accT")
                    nc.scalar.activation(
                        out=accT[:], in_=pT[:],
                        func=mybir.ActivationFunctionType.Copy)
                    pM = ppool.tile([Dl, 128], f32, tag="pM")
                    nc.tensor.matmul(pM[:], Wt[l][:], accT[:], start=True, stop=True)
                    hT = spool.tile([Dl, 128], f32, tag="hT")
                    if l < 2:
                        nc.scalar.activation(
                            out=hT[:], in_=pM[:],
                            func=mybir.ActivationFunctionType.Tanh,
                            bias=bt[l][:])
                    else:
                        nc.vector.tensor_scalar_add(hT[:], pM[:], bt[l][:])
                    p2 = ppool.tile([128, Dl], f32, tag="p2")
                    nc.tensor.transpose(p2[:], hT[:], ident[:Dl, :Dl])
                    res = spool.tile([128, Dl], f32, tag="res")
                    if l < 2:
                        nc.vector.tensor_scalar_mul(res[:], p2[:], dinv_t[:, b:b + 1])
                        nc.sync.dma_start(out=agins[l][b * 128:(b + 1) * 128, :], in_=res[:])
                    else:
                        nc.vector.tensor_copy(res[:], p2[:])
                        nc.sync.dma_start(out=z_ext[b * 128:(b + 1) * 128, :], in_=res[:])

                if l < 2:
                    nc.gpsimd.collective_compute(
                        "AllGather", mybir.AluOpType.bypass,
                        replica_groups=[list(range(C))],
                        ins=[agins[l][:]], outs=[tables[l + 1][:]])

    nc.finalize()
    return nc


def kernel(x, edge_index, W1, b1, W2, b2, W3, b3):
    global _last_results
    import os
    from concourse.bass_utils import run_bass_kernel_spmd

    meta, host = _prep(x, edge_index, W1, b1, W2, b2, W3, b3)
    nc = _build(meta)

    in_maps = []
    for c in range(C):
        in_maps.append({
            "x_table": host["x_table"],
            "idxL": host["idxL"][c], "idxH": host["idxH"][c],
            "idxR": host["idxR"][c], "dinv_t": host["dinv_t"][c],
            "W1": host["W1"], "W2": host["W2"], "W3": host["W3"],
            "b1": host["b1"], "b2": host["b2"], "b3": host["b3"],
        })
    res = run_bass_kernel_spmd(
        nc, in_maps, list(range(C)),
        trace=bool(int(os.environ.get("GCN_TRACE", "0"))))
    _last_results = res

    q0 = host["q0"]
    z = np.empty((N, DOUT), np.float32)
    for c in range(C):
        nodes = np.arange(c * NPC, (c + 1) * NPC)
        z[nodes] = res.results[c]["z"][q0[nodes]]
    return z

